# revision 14
# baseline (speedup 1.0000x reference)
"""BrainGCN Trainium2 kernel (8 NeuronCores, Bass/Tile).

Model (PyG-style GCNConv x2 + 2 FC layers):
    h = tanh(gcn(x,  W1, b1)); h = tanh(gcn(h, W2, b2))
    h = tanh(h @ W3 + b3);      out = h @ W4 + b4

gcn(x, W, b) = (agg + x * dinv^2) @ W + b  with
    agg[d] = sum_{e:(s,d)} dinv[s]*dinv[d] * x[s]        (by linearity we
aggregate raw feature rows first, then apply W once per node).

Distribution: dst-nodes are split into 8 contiguous blocks (one per core).
Each core aggregates its own dst block; self-loops are folded in via a
host-built diagonal (sdiag).  The cross-core exchange is an AllGather of
the h1 shards, chunked in two so the first chunk overlaps layer 1's tail.

Device-side scatter-add: edges are sorted by dst into 128-wide dst windows.
Host-built S tiles (smat) give S[e, d_local] = coef[e] * onehot(d_local[e]);
the PE accumulates aggT[feat, dst] += E_tile^T @ S into PSUM.  Gathers of
the 256-byte feature rows run on dma_gather with int16 indices; the gather
tables (x for L1, h1 for L2) are laid out in DRAM in two 25000-row halves,
with x rows permuted to match the AllGather's output interleave so both
layers share one set of index arrays.
"""

import numpy as np

# ---------------------------------------------------------------- constants
N_NODES = 50000
N_CORES = 8
F_IN, H1D, H2D, H3D, OUTD = 128, 128, 64, 64, 1
WIN = 128          # dst window width (psum free dim of the scatter matmul)
NPC = N_NODES // N_CORES          # 6250 dst nodes per core
QSH = NPC // 2                    # 3125: h1-shard chunk (AllGather chunk)
HALF = N_CORES * QSH              # 25000-row gather-table half
G_WINDOWS = 5      # dst windows per gather chunk (per half)
NCHUNK = 512       # fc-layer column chunk (one PSUM bank of f32)
N_QUEUES = 4


def _cdiv(a, b):
    return -(-a // b)


def _rup(a, b):
    return _cdiv(a, b) * b


# ------------------------------------------------------------------ planning
class Plan:
    pass


def make_plan(edge_index, n_nodes=N_NODES, n_cores=N_CORES,
              g_windows=G_WINDOWS, win=WIN):
    """Host-side graph preprocessing -> static schedule + per-core arrays."""
    src = np.asarray(edge_index[0]).astype(np.int64)
    dst = np.asarray(edge_index[1]).astype(np.int64)

    npc = n_nodes // n_cores
    assert npc * n_cores == n_nodes
    n_win = _cdiv(npc, win)
    n_half = 2
    half = n_nodes // 2
    assert half <= 32767

    deg = np.bincount(dst, minlength=n_nodes).astype(np.float64) + 1.0
    dinv = 1.0 / np.sqrt(deg)

    s_all, d_all = src, dst
    coef = (dinv[s_all] * dinv[d_all]).astype(np.float32)

    core = d_all // npc
    w = (d_all % npc) // win
    # gather-table half of a src node: which half of its owner's shard.
    # table position within the half: owner_core * QSH + (s % QSH).
    h = (s_all % npc) // QSH
    pos = (s_all // npc) * QSH + (s_all % QSH)

    # per (core, w, h) counts -> static caps shared by all cores
    gid = (core * n_win + w) * n_half + h
    counts = np.bincount(gid, minlength=n_cores * n_win * n_half)
    counts = counts.reshape(n_cores, n_win, n_half)
    caps = counts.max(axis=0)                     # [n_win, n_half] slots

    # window groups (gather chunks)
    wgroups = [list(range(i, min(i + g_windows, n_win)))
               for i in range(0, n_win, g_windows)]

    # static slot offsets, in [wg][h][w] order
    off = {}
    calls = []        # (wg_index, h, slot_off, n_slots)
    posn = 0
    for gi, wg in enumerate(wgroups):
        for hh in range(n_half):
            call_off = posn
            for ww in wg:
                off[(ww, hh)] = posn
                posn += int(caps[ww, hh])
            posn = _rup(posn, 128)                 # trailing pad per call
            calls.append((gi, hh, call_off, posn - call_off))
    S = posn                                       # total slots (mult of 128)
    assert S % 128 == 0
    T = S // 128                                   # total tiles

    # per-window tile lists: [(h, tile_local_in_call, tile_global)]
    win_tiles = []
    for ww in range(n_win):
        tiles = []
        for hh in range(n_half):
            gi = ww // g_windows
            call_off = next(c[2] for c in calls if c[0] == gi and c[1] == hh)
            o, cp = off[(ww, hh)], int(caps[ww, hh])
            if cp == 0:
                continue
            t_first = (o - call_off) // 128
            t_last = (o + cp - 1 - call_off) // 128
            for t in range(t_first, t_last + 1):
                tiles.append((hh, t, call_off // 128 + t))
        win_tiles.append(tiles)

    # ------- per-core arrays
    wg_of_w = np.array([ww // g_windows for ww in range(n_win)])
    order_key = (((core * len(wgroups) + wg_of_w[w]) * n_half + h) * n_win + w)
    order = np.argsort(order_key, kind="stable")
    s_o, d_o, c_o = s_all[order], d_all[order], coef[order]
    core_o, w_o, h_o = core[order], w[order], h[order]
    pos_o = pos[order]

    comb = (core_o * n_win + w_o) * n_half + h_o
    posi = np.arange(comb.size, dtype=np.int64)
    is_start = np.ones(comb.size, dtype=bool)
    if comb.size > 1:
        is_start[1:] = comb[1:] != comb[:-1]
    rank = posi - np.maximum.accumulate(np.where(is_start, posi, 0))

    static_off = np.zeros((n_win, n_half), dtype=np.int64)
    for ww in range(n_win):
        for hh in range(n_half):
            static_off[ww, hh] = off[(ww, hh)]
    slot = static_off[w_o, h_o] + rank

    idx16 = np.zeros((n_cores, S), dtype=np.int16)
    dloc = np.zeros((n_cores, S), dtype=np.int64)
    cof = np.zeros((n_cores, S), dtype=np.float32)
    ci = core_o.astype(np.int64)
    idx16[ci, slot] = pos_o.astype(np.int16)
    dloc[ci, slot] = d_o - ci * npc - w_o * win
    cof[ci, slot] = c_o

    p = Plan()
    p.n_nodes, p.n_cores, p.npc = n_nodes, n_cores, npc
    p.win, p.n_win, p.half, p.n_half = win, n_win, half, n_half
    p.win_sizes = [min(win, npc - ww * win) for ww in range(n_win)]
    p.wgroups, p.calls, p.win_tiles = wgroups, calls, win_tiles
    p.S, p.T = S, T
    # idx layout for the device: [128, S/16] (16-row wrap, replicated x8)
    p.idx_arr = np.ascontiguousarray(
        np.tile(idx16.reshape(n_cores, S // 16, 16).transpose(0, 2, 1),
                (1, 8, 1)))

    # window owner of each static slot (pad slots: -1)
    w_of_slot = np.full(S, -1, dtype=np.int64)
    for ww in range(n_win):
        for hh in range(n_half):
            o = off[(ww, hh)]
            w_of_slot[o:o + int(caps[ww, hh])] = ww

    # processing-sequence entries: for each wgroup, its windows' tiles.
    ents = []                     # (ww, hh, lt, gt)
    wg_ents = []                  # (ent_off, n_ents) per wgroup
    for gi, wg in enumerate(wgroups):
        e0 = len(ents)
        for ww in wg:
            for (hh, lt, gt) in win_tiles[ww]:
                ents.append((ww, hh, lt, gt))
        wg_ents.append((e0, len(ents) - e0))
    p.ents, p.wg_ents = ents, wg_ents
    NE = len(ents)

    # host-built S tiles in sequence order
    smat = np.zeros((n_cores, 128, NE * win), dtype=np.float16)
    cidx = np.arange(n_cores)[:, None]
    for q, (ww, hh, lt, gt) in enumerate(ents):
        sl = np.arange(gt * 128, gt * 128 + 128)
        m = w_of_slot[sl] == ww
        rows = np.where(m)[0]
        if rows.size == 0:
            continue
        smat[cidx, rows[None, :],
             q * win + dloc[:, sl[m]]] = cof[:, sl[m]]
    p.smat = np.ascontiguousarray(smat)
    p.NE = NE

    # self-loop diagonal: sdiag[c, p, w*win + j] = (p==j)*dinv^2[global node]
    d2 = (dinv * dinv).astype(np.float32)
    sdiag = np.zeros((n_cores, 128, n_win * win), dtype=np.float16)
    for c in range(n_cores):
        for ww in range(n_win):
            wsz = min(win, npc - ww * win)
            g0 = c * npc + ww * win
            sdiag[c, np.arange(wsz), ww * win + np.arange(wsz)] = d2[g0:g0 + wsz]
    p.sdiag = sdiag

    # x-table row permutation: node s -> table row h(s)*HALF + pos(s)
    s_ids = np.arange(n_nodes, dtype=np.int64)
    p.xrow = ((s_ids % npc) // QSH) * (n_cores * QSH) \
        + (s_ids // npc) * QSH + (s_ids % QSH)

    return p


# ------------------------------------------------------------------- program
def build_program(p, debug=False, n_queues=N_QUEUES, scratch=32768,
                  edge_dt="float16"):
    import concourse.bacc as bacc
    import concourse.mybir as mybir
    import concourse.tile as tile
    from concourse.masks import make_identity

    f32 = mybir.dt.float32
    edt = getattr(mybir.dt, edge_dt)
    i16 = mybir.dt.int16
    AF = mybir.ActivationFunctionType
    OP = mybir.AluOpType

    nc = bacc.Bacc("TRN2", target_bir_lowering=False, debug=debug,
                   num_devices=p.n_cores, num_swdge_queues=n_queues,
                   dynamic_dma_scratch_size=scratch)

    # weight pack: [128, 261] f32
    # cols 0:128 W1 | 128:192 W2 | 192:256 W3 (rows 0:64) | 256 W4 (rows 0:64)
    # 257 b1 | 258 b2 (rows 0:64) | 259 b3 (rows 0:64) | 260 b4 (row 0)
    NWP = 261
    x_d = nc.dram_tensor("xg", [p.n_nodes, F_IN], edt, kind="ExternalInput")
    idx_d = nc.dram_tensor("midx", [128, p.S // 16], i16, kind="ExternalInput")
    smat_d = nc.dram_tensor("smat", [128, p.NE * p.win], edt,
                            kind="ExternalInput")
    sdiag_d = nc.dram_tensor("sdiag", [128, p.n_win * p.win], edt,
                             kind="ExternalInput")
    xown_d = nc.dram_tensor("xown", [p.npc, F_IN], edt, kind="ExternalInput")
    wp_d = nc.dram_tensor("wpack", [128, NWP], f32, kind="ExternalInput")
    wp16_d = nc.dram_tensor("wpack16", [64, 65], edt, kind="ExternalInput")
    out_d = nc.dram_tensor("out", [p.npc, OUTD], f32, kind="ExternalOutput")

    h1_shard = nc.dram_tensor("h1_shard", [p.npc, H1D], edt)
    hv_d = nc.dram_tensor("hv", [p.n_nodes, H1D], edt, addr_space="Shared")

    with tile.TileContext(nc) as tc:
        with (
            tc.tile_pool(name="const", bufs=1) as cpool,
            tc.tile_pool(name="gather", bufs=2) as gpool,
            tc.tile_pool(name="sel", bufs=2) as spool,
            tc.tile_pool(name="work", bufs=3) as wpool,
            tc.tile_pool(name="persist", bufs=1) as ppool,
            tc.tile_pool(name="psum", bufs=2, space="PSUM") as pspool,
            tc.tile_pool(name="psumfc", bufs=1, space="PSUM") as pfpool,
        ):
            # ---- constants / metadata to SBUF
            wp_s = cpool.tile([128, NWP], f32)
            nc.sync.dma_start(wp_s[:], wp_d[:, :])
            wp16_s = cpool.tile([64, 65], edt)
            nc.sync.dma_start(wp16_s[:], wp16_d[:, :])
            idx_s = cpool.tile([128, p.S // 16], i16)
            nc.sync.dma_start(idx_s[:], idx_d[:, :])
            sdiag_s = cpool.tile([128, p.n_win * p.win], edt)
            nc.sync.dma_start(sdiag_s[:], sdiag_d[:, :])
            ident = cpool.tile([128, 128], edt)
            make_identity(nc, ident[:])

            w1_s = wp_s[:, 0:128]
            w2_s = wp_s[:, 128:192]
            w3_s = wp16_s[:, 0:64]
            w4_s = wp16_s[:, 64:65]
            b1_s = wp_s[:, 257:258]
            b2_s = wp_s[:64, 258:259]
            b3_s = wp_s[:64, 259:260]
            b4_s = wp_s[0:1, 260:261]

            h2T = ppool.tile([H2D, p.npc], edt)
            h1keep = ppool.tile([128, p.n_win * H1D], edt)

            call_seq = [0]

            def issue_gathers(gi, table_ap_fn):
                bufs = {}
                for (cgi, hh, call_off, n_call) in p.calls:
                    if cgi != gi or n_call == 0:
                        continue
                    gb = gpool.tile([128, (n_call // 128) * F_IN], edt,
                                    tag=f"gb{hh}")
                    out3d = gb[:].rearrange("q (t e) -> q t e", e=F_IN)
                    nc.gpsimd.dma_gather(
                        out_ap=out3d,
                        in_ap=table_ap_fn(hh),
                        idxs_ap=idx_s[:, call_off // 16:
                                      (call_off + n_call) // 16],
                        num_idxs=n_call,
                        num_idxs_reg=n_call,
                        elem_size=F_IN,
                        single_packet=False,
                        queue_num=call_seq[0] % n_queues,
                    )
                    call_seq[0] += 1
                    bufs[hh] = gb
                return bufs

            def stream_smat(gi):
                e0g, neg = p.wg_ents[gi]
                sbf = spool.tile([128, neg * p.win], edt, tag="sbf")
                nc.sync.dma_start(
                    sbf[:], smat_d[:, e0g * p.win:(e0g + neg) * p.win])
                return sbf

            def scatter_agg(ww, bufs, sbf, e0g, q0, self_lhsT):
                """Accumulate the window's aggregation into a PSUM tile."""
                tiles = p.win_tiles[ww]
                wsz = p.win_sizes[ww]
                pag = pspool.tile([128, p.win], f32, tag="pag")
                q = q0
                for k, (hh, lt, gt) in enumerate(tiles):
                    nc.tensor.matmul(
                        pag[:],
                        lhsT=bufs[hh][:, lt * F_IN:(lt + 1) * F_IN],
                        rhs=sbf[:, (q - e0g) * p.win:(q - e0g + 1) * p.win],
                        start=(k == 0), stop=False)
                    q += 1
                nc.tensor.matmul(
                    pag[:], lhsT=self_lhsT[:wsz, :],
                    rhs=sdiag_s[:wsz, ww * p.win:(ww + 1) * p.win],
                    start=(not tiles), stop=True)
                return pag

            def ag_chunk(k):
                nc.gpsimd.collective_compute(
                    "AllGather", mybir.AluOpType.bypass,
                    replica_groups=[list(range(p.n_cores))],
                    ins=[h1_shard[k * QSH:(k + 1) * QSH, :]],
                    outs=[hv_d[k * p.half:(k + 1) * p.half, :]])

            # ---------------- layer 1: x -> h1_shard + h1keep --------------
            def l1_table(hh):
                return x_d[hh * p.half:(hh + 1) * p.half, :]

            for gi, wg in enumerate(p.wgroups):
                bufs = issue_gathers(gi, l1_table)
                if gi == 8:
                    ag_chunk(0)     # h1 rows [0:QSH] are done by wg 4
                sbf = stream_smat(gi)
                xo = wpool.tile([128, len(wg) * F_IN], edt, tag="xo")
                xo3 = xo[:].rearrange("q (t e) -> q t e", e=F_IN)
                for jj, ww in enumerate(wg):
                    wsz = p.win_sizes[ww]
                    nc.sync.dma_start(
                        xo3[:wsz, jj, :],
                        xown_d[ww * p.win:ww * p.win + wsz, :])
                e0g = p.wg_ents[gi][0]
                q = e0g
                for ww in wg:
                    jj = ww - wg[0]
                    wsz = p.win_sizes[ww]
                    pag = scatter_agg(ww, bufs, sbf, e0g, q,
                                      xo[:, jj * F_IN:(jj + 1) * F_IN])
                    q += len(p.win_tiles[ww])
                    aggT = wpool.tile([128, p.win], f32, tag="aggT")
                    nc.vector.tensor_copy(aggT[:], pag[:])
                    ph = pspool.tile([128, p.win], f32, tag="ph")
                    nc.tensor.matmul(ph[:], lhsT=w1_s, rhs=aggT[:],
                                     start=True, stop=True)
                    h16 = wpool.tile([128, p.win], edt, tag="h16")
                    nc.scalar.activation(h16[:], ph[:], AF.Tanh, bias=b1_s)
                    pt = pspool.tile([128, p.win], edt, tag="pt")
                    nc.tensor.transpose(pt[:], h16[:], ident[:])
                    hk = h1keep[:, ww * H1D:(ww + 1) * H1D]
                    nc.vector.tensor_copy(hk, pt[:])
                    nc.sync.dma_start(
                        h1_shard[ww * p.win:ww * p.win + wsz, :],
                        hk[:wsz, :])

            ag_chunk(1)

            # ---------------- layer 2: hv -> h2T (on-chip, feat-major) ----
            def l2_table(hh):
                return hv_d[hh * p.half:(hh + 1) * p.half, :]

            for gi, wg in enumerate(p.wgroups):
                bufs = issue_gathers(gi, l2_table)
                sbf = stream_smat(gi)
                e0g = p.wg_ents[gi][0]
                q = e0g
                for ww in wg:
                    wsz = p.win_sizes[ww]
                    pag = scatter_agg(ww, bufs, sbf, e0g, q,
                                      h1keep[:, ww * H1D:(ww + 1) * H1D])
                    q += len(p.win_tiles[ww])
                    aggT = wpool.tile([128, p.win], f32, tag="aggT")
                    nc.vector.tensor_copy(aggT[:], pag[:])
                    ph = pspool.tile([128, p.win], f32, tag="ph")
                    nc.tensor.matmul(ph[:H2D, :], lhsT=w2_s, rhs=aggT[:],
                                     start=True, stop=True)
                    nc.scalar.activation(
                        h2T[:, ww * p.win:ww * p.win + wsz],
                        ph[:H2D, :wsz], AF.Tanh, bias=b2_s)

            # ---------------- fc layers on the dst shard ------------------
            for c0 in range(0, p.npc, NCHUNK):
                cs = min(NCHUNK, p.npc - c0)
                p3 = pfpool.tile([H3D, NCHUNK], f32, tag="p3")
                nc.tensor.matmul(p3[:, :cs], lhsT=w3_s,
                                 rhs=h2T[:, c0:c0 + cs],
                                 start=True, stop=True)
                h3 = wpool.tile([H3D, NCHUNK], edt, tag="h3")
                nc.scalar.activation(h3[:, :cs], p3[:, :cs], AF.Tanh,
                                     bias=b3_s)
                p4 = pfpool.tile([OUTD, NCHUNK], f32, tag="p4")
                nc.tensor.matmul(p4[:, :cs], lhsT=w4_s, rhs=h3[:, :cs],
                                 start=True, stop=True)
                ob = wpool.tile([OUTD, NCHUNK], f32, tag="ob")
                nc.vector.tensor_scalar(
                    out=ob[:, :cs], in0=p4[:, :cs],
                    scalar1=b4_s, scalar2=None, op0=OP.add)
                nc.sync.dma_start(out_d[c0:c0 + cs, :], ob[0:1, :cs])

    nc.compile()
    return nc


def make_in_maps(p, inputs, edge_dt="float16"):
    np_edt = dict(float32=np.float32, float16=np.float16)[edge_dt]
    x = np.asarray(inputs["x"]).astype(np_edt)
    xg = np.empty_like(x)
    xg[p.xrow] = x                       # table-order layout (two halves)
    xg = np.ascontiguousarray(xg)

    wpack = np.zeros((128, 261), dtype=np.float32)
    wpack[:, 0:128] = np.asarray(inputs["W1"], dtype=np.float32)
    wpack[:, 128:192] = np.asarray(inputs["W2"], dtype=np.float32)
    wpack[:, 257] = np.asarray(inputs["b1"], dtype=np.float32)
    wpack[:64, 258] = np.asarray(inputs["b2"], dtype=np.float32)
    wpack[:64, 259] = np.asarray(inputs["b3"], dtype=np.float32)
    wpack[0, 260] = np.asarray(inputs["b4"], dtype=np.float32)[0]
    wpack16 = np.zeros((64, 65), dtype=np_edt)
    wpack16[:, 0:64] = np.asarray(inputs["W3"], dtype=np_edt)
    wpack16[:, 64:65] = np.asarray(inputs["W4"], dtype=np_edt)

    maps = []
    for c in range(p.n_cores):
        maps.append({
            "xg": xg,
            "midx": p.idx_arr[c],
            "smat": p.smat[c].astype(np_edt, copy=False),
            "sdiag": p.sdiag[c].astype(np_edt, copy=False),
            "xown": x[c * p.npc:(c + 1) * p.npc],
            "wpack": wpack,
            "wpack16": wpack16,
        })
    return maps


_CACHE = {}


def kernel(_trace=False, **inputs):
    from concourse.bass_utils import run_bass_kernel_spmd

    edge_index = np.asarray(inputs["edge_index"])
    p = make_plan(edge_index)
    key = (p.S, tuple(int(c[3]) for c in p.calls))
    if key not in _CACHE:
        _CACHE[key] = build_program(p)
    nc = _CACHE[key]
    res = run_bass_kernel_spmd(nc, make_in_maps(p, inputs),
                               core_ids=list(range(p.n_cores)),
                               trace=_trace)
    out = np.concatenate([res.results[c]["out"] for c in range(p.n_cores)],
                         axis=0)
    if _trace:
        return out, res
    return out
